# revision 6
# baseline (speedup 1.0000x reference)
"""Trainium2 Bass kernel for 6-layer CROWN/DeepPoly-style backsubstitution.

Problem: L=6, D=2048. Sequential loop of 5 composition steps, each doing
4 masked DxDxD matmuls plus small concretization matvecs, then a final
concretization against layer-0 input bounds.

Sharding: tensor-parallel over the row dimension of the running symbolic
bound matrices Wu/Wl — core r owns output rows [r*256, (r+1)*256). Row
shards compose independently (prev-layer weights are broadcast inputs),
so there are NO collectives; the only gather is host-side concatenation
of the per-core 256-row bound vectors.

Formulation tricks:
  * The running state is kept TRANSPOSED (Vu = Wu^T as [2048, 256]) so
    the composition matmuls use prev-layer weights directly as the
    stationary lhsT operand — no on-device transposes anywhere.
  * The lower-bound state is kept NEGATED (Vl' = -Wl^T). Then upper and
    lower compositions/concretizations have identical structure and the
    state packs as SA = [pos(Vu) | pos(Vl')], SB = [neg(Vu) | neg(Vl')]
    tiles of [128, 512]: one relu / one min-with-0 per tile, and the
    best-bound update is a single elementwise MIN on [1, 512].
  * Matmuls run in bfloat16: on real TRN2 a 512-row bf16 matmul takes
    ~204 ns (weight load fully hidden behind streaming) vs ~327 ns for
    float32r (load not hidden), and weight DMA bytes halve. fp32 PSUM
    accumulation keeps the error ~1e-2 (max-abs), inside the 2e-2 gate.
  * Only the masked state is materialized (raw Vu/Vl never stored);
    masking is fused with the PSUM->SBUF drain.
"""

import ml_dtypes
import numpy as np

import concourse.bass as bass
import concourse.bacc as bacc
import concourse.tile as tile
from concourse import mybir
from concourse.bass_utils import run_bass_kernel_spmd

L = 6
D = 2048
N_CORES = 8
MSH = D // N_CORES          # 256 rows per core
KT = D // 128               # 16 k-tiles
NQ = 4                      # m'-quarters per iteration
KPC = 4                     # k-tiles per weight DMA chunk
N_CHUNKS = KT // KPC        # weight DMA chunks per quarter
WCH_BUFS = 3                # weight chunk buffers (KPC*16KB/partition each)
NSWEEP = L                  # 6 concretization sweeps (iters t=0..4 + final)
NITER = L - 1               # 5 composition iterations

F32 = mybir.dt.float32
BF16 = mybir.dt.bfloat16
NPBF16 = ml_dtypes.bfloat16

# Filled by kernel() when _trace=True (used by test.py)
LAST_RESULTS = None

_CACHED_NC = None


def _vec_slice(t, ab, k):
    """Column slice of the packed vec-lhsT tile for sweep t, side ab, k-tile k."""
    j = ((t * 2 + ab) * KT + k) * 2
    return slice(j, j + 2)


def _build_nc(bench_reps=1, parts=("sweep", "comp")):
    nc = bacc.Bacc("TRN2", target_bir_lowering=False, debug=False,
                   num_devices=N_CORES)

    WQ = nc.dram_tensor("WQ", [NITER, NQ, N_CHUNKS, 128, KPC * 1024], BF16,
                        kind="ExternalInput").ap()
    SAB0 = nc.dram_tensor("SAB0", [2, KT, 128, 512], BF16,
                          kind="ExternalInput").ap()
    VECS = nc.dram_tensor("VECS", [128, NSWEEP * 2 * KT * 2], BF16,
                          kind="ExternalInput").ap()
    BI = nc.dram_tensor("BI", [2, 512], F32, kind="ExternalInput").ap()
    OUT = nc.dram_tensor("OUT", [1, 512], F32, kind="ExternalOutput").ap()

    with tile.TileContext(nc) as tc:
        with (
            tc.tile_pool(name="state", bufs=1) as state_pool,
            tc.tile_pool(name="wchunk", bufs=WCH_BUFS) as wpool,
            tc.tile_pool(name="small", bufs=1) as spool,
            tc.tile_pool(name="tmp", bufs=2) as tpool,
            tc.tile_pool(name="qpsum", bufs=7, space="PSUM") as qpool,
            tc.tile_pool(name="vpsum", bufs=1, space="PSUM") as vpool,
        ):
            if bench_reps > 1:
                loop_ctx = tc.For_i(0, bench_reps, 1)
                loop_ctx.__enter__()

            # Persistent per-core state: two ping-pong sets of masked tiles.
            SA = [[state_pool.tile([128, 512], BF16, tag=f"sa{s}_{k}",
                                   name=f"sa{s}_{k}")
                   for k in range(KT)] for s in range(2)]
            SB = [[state_pool.tile([128, 512], BF16, tag=f"sb{s}_{k}",
                                   name=f"sb{s}_{k}")
                   for k in range(KT)] for s in range(2)]

            vecs = spool.tile([128, NSWEEP * 2 * KT * 2], BF16, tag="vecs")
            nc.scalar.dma_start(vecs[:], VECS[:])
            best = spool.tile([1, 512], F32, tag="best")
            nc.scalar.dma_start(best[:], BI[0:1, :])
            bb = spool.tile([1, 512], F32, tag="bb")
            nc.scalar.dma_start(bb[:], BI[1:2, :])

            for k in range(KT):
                nc.scalar.dma_start(SA[0][k][:], SAB0[0, k])
                nc.scalar.dma_start(SB[0][k][:], SAB0[1, k])

            def sweep(t, cur, update_bb):
                """Concretization (+ bias composition) sweep against state set `cur`."""
                pv = vpool.tile([2, 512], F32, tag="pv")
                for k in range(KT):
                    nc.tensor.matmul(pv[:], vecs[:, _vec_slice(t, 0, k)],
                                     SA[cur][k][:], start=(k == 0), stop=False)
                for k in range(KT):
                    nc.tensor.matmul(pv[:], vecs[:, _vec_slice(t, 1, k)],
                                     SB[cur][k][:], start=False, stop=(k == KT - 1))
                curt = tpool.tile([1, 512], F32, tag="curt")
                nc.vector.tensor_add(curt[:], pv[0:1, :], bb[:])
                nc.vector.tensor_tensor(best[:], best[:], curt[:],
                                        op=mybir.AluOpType.min)
                if update_bb:
                    # Engines may not address partition bases other than 0, so
                    # evacuate both psum rows at base 0 and shift row 1 down to
                    # partition 0 with a tiny SBUF->SBUF DMA.
                    s2 = tpool.tile([2, 512], F32, tag="s2")
                    nc.scalar.copy(s2[:], pv[0:2, :])
                    bdel = tpool.tile([1, 512], F32, tag="bdel")
                    nc.sync.dma_start(bdel[:], s2[1:2, :])
                    nc.vector.tensor_add(bb[:], bb[:], bdel[:])

            for t in range(NITER):
                cur, nxt = t % 2, 1 - t % 2
                if "comp" not in parts:
                    cur, nxt = 0, 1
                if "sweep" in parts:
                    sweep(t, cur, update_bb=True)
                if "comp" not in parts:
                    continue
                # Composition: [Vu_new | Vl'_new] = Wup^T @ SA + Wlo^T @ SB,
                # processed in 4 m'-quarters of 4 psum banks each.
                for q in range(NQ):
                    ps = [qpool.tile([128, 512], F32, tag="qp", name=f"qp_{t}_{q}_{j}")
                          for j in range(4)]
                    for c in range(N_CHUNKS):
                        wch = wpool.tile([128, KPC * 1024], BF16, tag="wch")
                        nc.sync.dma_start(wch[:], WQ[t, q, c])
                        for kk in range(KPC):
                            k = KPC * c + kk
                            for j in range(4):
                                nc.tensor.matmul(
                                    ps[j][:],
                                    wch[:, kk * 1024 + j * 128:kk * 1024 + (j + 1) * 128],
                                    SA[cur][k][:],
                                    start=(k == 0), stop=False)
                                nc.tensor.matmul(
                                    ps[j][:],
                                    wch[:, kk * 1024 + 512 + j * 128:kk * 1024 + 512 + (j + 1) * 128],
                                    SB[cur][k][:],
                                    start=False, stop=(k == KT - 1))
                    for j in range(4):
                        m = 4 * q + j
                        nc.scalar.activation(SA[nxt][m][:], ps[j][:],
                                             mybir.ActivationFunctionType.Relu)
                        nc.vector.tensor_scalar_min(SB[nxt][m][:], ps[j][:], 0.0)

            if "sweep" in parts:
                fin = NITER % 2 if "comp" in parts else 0
                sweep(NITER, fin, update_bb=False)
            nc.sync.dma_start(OUT[0:1, :], best[:])

            if bench_reps > 1:
                loop_ctx.__exit__(None, None, None)

    nc.compile()
    return nc


def _host_inputs(W_upper, W_lower, b_upper, b_lower, ub, lb):
    """Build the rearranged per-core input maps (all float32, C-contiguous)."""
    f = np.float32
    W_upper = np.asarray(W_upper, dtype=f)
    W_lower = np.asarray(W_lower, dtype=f)
    b_upper = np.asarray(b_upper, dtype=f)
    b_lower = np.asarray(b_lower, dtype=f)
    ub = np.asarray(ub, dtype=f)
    lb = np.asarray(lb, dtype=f)

    # Weights for iterations t=0..4 use layer i=4-t. Layout:
    # WQ[t, q, c, p, kk*1024 + h*512 + col] = W_h[i][(4c+kk)*128 + p, q*512 + col]
    A = W_upper[NITER - 1::-1].reshape(NITER, N_CHUNKS, KPC, 128, 4, 512)
    B = W_lower[NITER - 1::-1].reshape(NITER, N_CHUNKS, KPC, 128, 4, 512)
    T = np.stack([A, B], axis=5)                # [t, c, kk, p, q, h, col]
    WQ = np.ascontiguousarray(
        T.transpose(0, 4, 1, 3, 2, 5, 6).reshape(
            NITER, NQ, N_CHUNKS, 128, KPC * 1024).astype(NPBF16))

    # Packed vec-lhsT: V[t, ab, k, p, e]
    V = np.zeros((NSWEEP, 2, KT, 128, 2), dtype=f)
    for t in range(NITER):
        i = NITER - 1 - t
        V[t, 0, :, :, 0] = ub[i].reshape(KT, 128)
        V[t, 0, :, :, 1] = b_upper[i].reshape(KT, 128)
        V[t, 1, :, :, 0] = lb[i].reshape(KT, 128)
        V[t, 1, :, :, 1] = b_lower[i].reshape(KT, 128)
    V[NITER, 0, :, :, 0] = ub[0].reshape(KT, 128)
    V[NITER, 1, :, :, 0] = lb[0].reshape(KT, 128)
    VECS = np.ascontiguousarray(
        V.transpose(3, 0, 1, 2, 4).reshape(128, NSWEEP * 2 * KT * 2).astype(NPBF16))

    in_maps = []
    for r in range(N_CORES):
        rows = slice(r * MSH, (r + 1) * MSH)
        Vu = W_upper[L - 1][rows, :].T          # [2048, 256]
        Vln = -W_lower[L - 1][rows, :].T
        st = np.concatenate([Vu, Vln], axis=1)  # [2048, 512]
        SA0 = np.maximum(st, 0).reshape(KT, 128, 512)
        SB0 = np.minimum(st, 0).reshape(KT, 128, 512)
        SAB0 = np.ascontiguousarray(np.stack([SA0, SB0], axis=0).astype(NPBF16))

        BI = np.empty((2, 512), dtype=f)
        BI[0, :256] = ub[L - 1][rows]
        BI[0, 256:] = -lb[L - 1][rows]
        BI[1, :256] = b_upper[L - 1][rows]
        BI[1, 256:] = -b_lower[L - 1][rows]

        in_maps.append(dict(WQ=WQ, SAB0=SAB0, VECS=VECS, BI=BI))
    return in_maps


def kernel(W_upper, W_lower, b_upper, b_lower, ub, lb, _trace=False):
    global _CACHED_NC, LAST_RESULTS
    if _CACHED_NC is None:
        _CACHED_NC = _build_nc()
    nc = _CACHED_NC

    in_maps = _host_inputs(W_upper, W_lower, b_upper, b_lower, ub, lb)
    res = run_bass_kernel_spmd(nc, in_maps, list(range(N_CORES)), trace=_trace)
    LAST_RESULTS = res

    best_ub = np.empty(D, dtype=np.float32)
    best_lb = np.empty(D, dtype=np.float32)
    for r in range(N_CORES):
        o = res.results[r]["OUT"].reshape(512)
        best_ub[r * MSH:(r + 1) * MSH] = o[:256]
        best_lb[r * MSH:(r + 1) * MSH] = -o[256:]
    return best_ub, best_lb



# revision 12
# speedup vs baseline: 1.0142x; 1.0142x over previous
"""Trainium2 Bass kernel for 6-layer CROWN/DeepPoly-style backsubstitution.

Problem: L=6, D=2048. Sequential loop of 5 composition steps, each doing
4 masked DxDxD matmuls plus small concretization matvecs, then a final
concretization against layer-0 input bounds.

Sharding: tensor-parallel over the row dimension of the running symbolic
bound matrices Wu/Wl — core r owns output rows [r*256, (r+1)*256). Row
shards compose independently (prev-layer weights are broadcast inputs),
so there are NO collectives; the only gather is host-side concatenation
of the per-core 256-row bound vectors.

Formulation tricks:
  * The running state is kept TRANSPOSED (Vu = Wu^T as [2048, 256]) so
    the composition matmuls use prev-layer weights directly as the
    stationary lhsT operand — no on-device transposes anywhere.
  * The lower-bound state is kept NEGATED (Vl' = -Wl^T). Then upper and
    lower compositions/concretizations have identical structure and the
    state packs as SA = [pos(Vu) | pos(Vl')], SB = [neg(Vu) | neg(Vl')]
    tiles of [128, 512]: one relu / one min-with-0 per tile, and the
    best-bound update is a single elementwise MIN on [1, 512].
  * Matmuls run in bfloat16: on real TRN2 a 512-row bf16 matmul takes
    ~204 ns (weight load fully hidden behind streaming) vs ~327 ns for
    float32r (load not hidden), and weight DMA bytes halve. fp32 PSUM
    accumulation keeps the error ~1e-2 (max-abs), inside the 2e-2 gate.
  * One PSUM accumulation chain per output m'-tile (32 matmuls into one
    bank, then drain). HW-measured: interleaving 4 open accumulation
    groups costs ~494 ns/matmul vs ~208 ns for a single sequential
    chain, so the chain-per-tile order is 2.4x faster than round-robin.
  * Only the masked state is materialized (raw Vu/Vl never stored);
    masking is fused with the PSUM->SBUF drain.
"""

import ml_dtypes
import numpy as np

import concourse.bass as bass
import concourse.bacc as bacc
import concourse.tile as tile
from concourse import mybir
from concourse.bass_utils import run_bass_kernel_spmd

L = 6
D = 2048
N_CORES = 8
MSH = D // N_CORES          # 256 rows per core
KT = D // 128               # 16 k-tiles
WCH_BUFS = 3                # weight chunk buffers (8KB/partition each)
NSWEEP = L                  # 6 concretization sweeps (iters t=0..4 + final)
NITER = L - 1               # 5 composition iterations

F32 = mybir.dt.float32
BF16 = mybir.dt.bfloat16
NPBF16 = ml_dtypes.bfloat16

# Filled by kernel() when _trace=True (used by test.py)
LAST_RESULTS = None

_CACHED_NC = None


def _vec_slice(t, ab, k):
    """Column slice of the packed vec-lhsT tile for sweep t, side ab, k-tile k."""
    j = ((t * 2 + ab) * KT + k) * 2
    return slice(j, j + 2)


def _build_nc(bench_reps=1, parts=("sweep", "comp")):
    nc = bacc.Bacc("TRN2", target_bir_lowering=False, debug=False,
                   num_devices=N_CORES)

    WQ = nc.dram_tensor("WQ", [NITER, KT, 128, KT * 2 * 128], BF16,
                        kind="ExternalInput").ap()
    SAB0 = nc.dram_tensor("SAB0", [2, KT, 128, 512], BF16,
                          kind="ExternalInput").ap()
    VECS = nc.dram_tensor("VECS", [128, NSWEEP * 2 * KT * 2], BF16,
                          kind="ExternalInput").ap()
    BI = nc.dram_tensor("BI", [2, 512], F32, kind="ExternalInput").ap()
    OUT = nc.dram_tensor("OUT", [1, 512], F32, kind="ExternalOutput").ap()

    with tile.TileContext(nc) as tc:
        with (
            tc.tile_pool(name="state", bufs=1) as state_pool,
            tc.tile_pool(name="wchunk", bufs=WCH_BUFS) as wpool,
            tc.tile_pool(name="small", bufs=1) as spool,
            tc.tile_pool(name="tmp", bufs=2) as tpool,
            tc.tile_pool(name="qpsum", bufs=6, space="PSUM") as qpool,
            tc.tile_pool(name="vpsum", bufs=1, space="PSUM") as vpool,
        ):
            if bench_reps > 1:
                loop_ctx = tc.For_i(0, bench_reps, 1)
                loop_ctx.__enter__()

            # Persistent per-core state: two ping-pong sets of masked tiles.
            SA = [[state_pool.tile([128, 512], BF16, tag=f"sa{s}_{k}",
                                   name=f"sa{s}_{k}")
                   for k in range(KT)] for s in range(2)]
            SB = [[state_pool.tile([128, 512], BF16, tag=f"sb{s}_{k}",
                                   name=f"sb{s}_{k}")
                   for k in range(KT)] for s in range(2)]

            vecs = spool.tile([128, NSWEEP * 2 * KT * 2], BF16, tag="vecs")
            nc.scalar.dma_start(vecs[:], VECS[:])
            best = spool.tile([1, 512], F32, tag="best")
            nc.scalar.dma_start(best[:], BI[0:1, :])
            bb = spool.tile([1, 512], F32, tag="bb")
            nc.scalar.dma_start(bb[:], BI[1:2, :])

            for k in range(KT):
                nc.scalar.dma_start(SA[0][k][:], SAB0[0, k])
                nc.scalar.dma_start(SB[0][k][:], SAB0[1, k])

            def sweep(t, cur, update_bb):
                """Concretization (+ bias composition) sweep against state set `cur`."""
                pv = vpool.tile([2, 512], F32, tag="pv")
                for k in range(KT):
                    nc.tensor.matmul(pv[:], vecs[:, _vec_slice(t, 0, k)],
                                     SA[cur][k][:], start=(k == 0), stop=False)
                for k in range(KT):
                    nc.tensor.matmul(pv[:], vecs[:, _vec_slice(t, 1, k)],
                                     SB[cur][k][:], start=False, stop=(k == KT - 1))
                curt = tpool.tile([1, 512], F32, tag="curt")
                nc.vector.tensor_add(curt[:], pv[0:1, :], bb[:])
                nc.vector.tensor_tensor(best[:], best[:], curt[:],
                                        op=mybir.AluOpType.min)
                if update_bb:
                    # Engines may not address partition bases other than 0, so
                    # evacuate both psum rows at base 0 and shift row 1 down to
                    # partition 0 with a tiny SBUF->SBUF DMA.
                    s2 = tpool.tile([2, 512], F32, tag="s2")
                    nc.scalar.copy(s2[:], pv[0:2, :])
                    bdel = tpool.tile([1, 512], F32, tag="bdel")
                    nc.sync.dma_start(bdel[:], s2[1:2, :])
                    nc.vector.tensor_add(bb[:], bb[:], bdel[:])

            for t in range(NITER):
                cur, nxt = t % 2, 1 - t % 2
                if "comp" not in parts:
                    cur, nxt = 0, 1
                if "sweep" in parts:
                    sweep(t, cur, update_bb=True)
                if "comp" not in parts:
                    continue
                # Composition: [Vu_new | Vl'_new] = Wup^T @ SA + Wlo^T @ SB.
                # One 32-matmul accumulation chain per output m'-tile into a
                # single PSUM bank (interleaved accumulation groups measure
                # ~2.4x slower on HW), then fused masked drain.
                for m in range(KT):
                    wch = wpool.tile([128, KT * 2 * 128], BF16, tag="wch",
                                     name=f"wch_{t}_{m}")
                    nc.sync.dma_start(wch[:], WQ[t, m])
                    ps = qpool.tile([128, 512], F32, tag="qp", name=f"qp_{t}_{m}")
                    for k in range(KT):
                        nc.tensor.matmul(
                            ps[:], wch[:, (2 * k) * 128:(2 * k + 1) * 128],
                            SA[cur][k][:], start=(k == 0), stop=False)
                        nc.tensor.matmul(
                            ps[:], wch[:, (2 * k + 1) * 128:(2 * k + 2) * 128],
                            SB[cur][k][:], start=False, stop=(k == KT - 1))
                    nc.scalar.activation(SA[nxt][m][:], ps[:],
                                         mybir.ActivationFunctionType.Relu)
                    nc.vector.tensor_scalar_min(SB[nxt][m][:], ps[:], 0.0)

            if "sweep" in parts:
                fin = NITER % 2 if "comp" in parts else 0
                sweep(NITER, fin, update_bb=False)
            nc.sync.dma_start(OUT[0:1, :], best[:])

            if bench_reps > 1:
                loop_ctx.__exit__(None, None, None)

    nc.compile()
    return nc


def _host_inputs(W_upper, W_lower, b_upper, b_lower, ub, lb):
    """Build the rearranged per-core input maps (all float32, C-contiguous)."""
    f = np.float32
    W_upper = np.asarray(W_upper, dtype=f)
    W_lower = np.asarray(W_lower, dtype=f)
    b_upper = np.asarray(b_upper, dtype=f)
    b_lower = np.asarray(b_lower, dtype=f)
    ub = np.asarray(ub, dtype=f)
    lb = np.asarray(lb, dtype=f)

    # Weights for iterations t=0..4 use layer i=4-t. Layout:
    # WQ[t, m, p, (k*2 + h)*128 + c] = W_h[i][k*128 + p, m*128 + c]
    A = W_upper[NITER - 1::-1].reshape(NITER, KT, 128, KT, 128)
    B = W_lower[NITER - 1::-1].reshape(NITER, KT, 128, KT, 128)
    T = np.stack([A, B], axis=4)                # [t, k, p, m, h, c]
    WQ = np.ascontiguousarray(
        T.transpose(0, 3, 2, 1, 4, 5).reshape(
            NITER, KT, 128, KT * 2 * 128).astype(NPBF16))

    # Packed vec-lhsT: V[t, ab, k, p, e]
    V = np.zeros((NSWEEP, 2, KT, 128, 2), dtype=f)
    for t in range(NITER):
        i = NITER - 1 - t
        V[t, 0, :, :, 0] = ub[i].reshape(KT, 128)
        V[t, 0, :, :, 1] = b_upper[i].reshape(KT, 128)
        V[t, 1, :, :, 0] = lb[i].reshape(KT, 128)
        V[t, 1, :, :, 1] = b_lower[i].reshape(KT, 128)
    V[NITER, 0, :, :, 0] = ub[0].reshape(KT, 128)
    V[NITER, 1, :, :, 0] = lb[0].reshape(KT, 128)
    VECS = np.ascontiguousarray(
        V.transpose(3, 0, 1, 2, 4).reshape(128, NSWEEP * 2 * KT * 2).astype(NPBF16))

    in_maps = []
    for r in range(N_CORES):
        rows = slice(r * MSH, (r + 1) * MSH)
        Vu = W_upper[L - 1][rows, :].T          # [2048, 256]
        Vln = -W_lower[L - 1][rows, :].T
        st = np.concatenate([Vu, Vln], axis=1)  # [2048, 512]
        SA0 = np.maximum(st, 0).reshape(KT, 128, 512)
        SB0 = np.minimum(st, 0).reshape(KT, 128, 512)
        SAB0 = np.ascontiguousarray(np.stack([SA0, SB0], axis=0).astype(NPBF16))

        BI = np.empty((2, 512), dtype=f)
        BI[0, :256] = ub[L - 1][rows]
        BI[0, 256:] = -lb[L - 1][rows]
        BI[1, :256] = b_upper[L - 1][rows]
        BI[1, 256:] = -b_lower[L - 1][rows]

        in_maps.append(dict(WQ=WQ, SAB0=SAB0, VECS=VECS, BI=BI))
    return in_maps


def kernel(W_upper, W_lower, b_upper, b_lower, ub, lb, _trace=False):
    global _CACHED_NC, LAST_RESULTS
    if _CACHED_NC is None:
        _CACHED_NC = _build_nc()
    nc = _CACHED_NC

    in_maps = _host_inputs(W_upper, W_lower, b_upper, b_lower, ub, lb)
    res = run_bass_kernel_spmd(nc, in_maps, list(range(N_CORES)), trace=_trace)
    LAST_RESULTS = res

    best_ub = np.empty(D, dtype=np.float32)
    best_lb = np.empty(D, dtype=np.float32)
    for r in range(N_CORES):
        o = res.results[r]["OUT"].reshape(512)
        best_ub[r * MSH:(r + 1) * MSH] = o[:256]
        best_lb[r * MSH:(r + 1) * MSH] = -o[256:]
    return best_ub, best_lb

